# revision 18
# baseline (speedup 1.0000x reference)
"""DynaLoRALinear Trainium2 kernel (v6).

Data-parallel over batch B across 8 NeuronCores (one sample per core).
Per core:
  - router:  logits = pooled @ (W_r @ gating_W).T  computed as a sharded
    partial (each core contracts over a 512-wide slice of D) + AllReduce.
  - gate weights from expert_scores ranks + module_prob>0.5 branch select.
  - base:    out = x_b @ W_base.T + b_base   (bf16 matmuls, fp32 PSUM accum)
  - lora:    t = x_b @ A_cat.T (fused into chunk-0 k-loop), then
             out += t @ (B_cat * gate).T

Performance structure (see git history for the evolution):
  - chunks 1-7: K=64 lo/hi row-split matmuls on concurrent PE row groups
    (hides the stationary swap), 4-way row-tiled K=32 lora combines,
    pure-DVE PSUM release, stores on the scalar ring.
  - chunk 0 (runs in the pre-GPIO-throttle full-clock window): all inputs
    (x mg0 tiles, W chunk-0, gating_W, router smalls) issued upfront on
    three separate rings -- jit interleaving created false cross-stream
    waits through the 8 round-robin DMA completion lanes.
  - W prefetch for the next chunk is 4 big [128,8,512] DMAs on the scalar
    ring (fewer lane slots, no ACT-FIFO head-blocking of evictions).
  - router tail / part-B PE ops are emitted only at group boundaries:
    mid-group emission deadlocks (PSUM pool rotation reuses banks whose
    eviction sits behind the emitted op in the strict PE FIFO), and their
    DVE dependencies are pre-staged so the PE transposes never idle.
"""

import sys
import types

import numpy as np

B, L, D, E, R, NMOD = 8, 2048, 4096, 4, 8, 7
N_CORES = 8
DSH = D // N_CORES  # 512: per-core slice of D for the router shard
ER = E * R          # 32
KT = D // 128       # 32 k-tiles
MT = L // 128       # 16 m-tiles


def _to_bf16(a) -> np.ndarray:
    import ml_dtypes
    return np.ascontiguousarray(np.asarray(a, dtype=np.float32)
                                .astype(ml_dtypes.bfloat16))


def _install_profile_hook():
    """Make bass_utils' trace path importable (no-op if already present)."""
    try:
        import antenv.axon_hooks  # noqa: F401
        return
    except ImportError:
        pass
    try:
        import antenv
    except ImportError:
        return
    mod = types.ModuleType("antenv.axon_hooks")
    mod._hook = None
    mod.set_axon_ntff_profile_hook = lambda h: setattr(mod, "_hook", h)
    mod.get_axon_ntff_profile_hook = lambda: mod._hook
    sys.modules["antenv.axon_hooks"] = mod
    antenv.axon_hooks = mod
    try:
        from trn_agent_boot.trn_boot import _ntff_profile_via_ctypes
        hook = _ntff_profile_via_ctypes("/opt/axon/libaxon_pjrt.so")
        if hook is not None:
            mod.set_axon_ntff_profile_hook(hook)
    except Exception:
        pass


_PROGRAM_CACHE = {}


def _build_program(k: int, module_idx: int, has_bias: bool):
    import concourse.mybir as mybir
    import concourse.tile as tile
    from concourse import bacc
    from concourse.masks import make_identity

    f32 = mybir.dt.float32
    bf16 = mybir.dt.bfloat16
    alu = mybir.AluOpType
    act_fn = mybir.ActivationFunctionType

    k_lo = max(1, k // 2)

    nc = bacc.Bacc("TRN2", target_bir_lowering=False, debug=False,
                   num_devices=N_CORES)

    # --- DRAM I/O -------------------------------------------------------
    xT = nc.dram_tensor("xT", [D, L], bf16, kind="ExternalInput")
    WbT = nc.dram_tensor("WbT", [D, D], bf16, kind="ExternalInput")
    gw = nc.dram_tensor("gw", [D, DSH], bf16, kind="ExternalInput")
    # pre-packed on host to [128, ...] so the DMA is contiguous per
    # partition (the "(a p) m -> p a m" rearrange pattern costs thousands
    # of sub-64B descriptors)
    WrT = nc.dram_tensor("WrT", [128, KT * NMOD], bf16,
                         kind="ExternalInput")
    pooledT = nc.dram_tensor("pooledT", [128, 4 * B], f32,
                             kind="ExternalInput")
    scores_f = nc.dram_tensor("scores_f", [1, E * B], f32,
                              kind="ExternalInput")
    A_rhs = nc.dram_tensor("A_rhs", [128, KT * ER], bf16,
                           kind="ExternalInput")
    B_cat = nc.dram_tensor("B_cat", [ER, D], f32, kind="ExternalInput")
    b_row = nc.dram_tensor("b_row", [1, D], f32, kind="ExternalInput")
    msel = nc.dram_tensor("msel", [ER, E * B], f32, kind="ExternalInput")
    out = nc.dram_tensor("out", [L, D], bf16, kind="ExternalOutput")

    with tile.TileContext(nc) as tc:
        with (
            tc.tile_pool(name="const", bufs=1) as const_pool,
            tc.tile_pool(name="gatep", bufs=1) as gate_pool,
            tc.tile_pool(name="rsb", bufs=1) as rsb,
            # big W tiles [128, 8, 512] bf16 = 8KB/partition; 4 = one
            # chunk; 8 = current + prefetched next
            tc.tile_pool(name="wpool", bufs=8) as wpool,
            # chunk-0 W tiles, loaded individually upfront
            tc.tile_pool(name="w0pool", bufs=KT) as w0pool,
            tc.tile_pool(name="xpool", bufs=14) as xpool,
            # mg0 xs tiles: chunk-0's full set upfront; 8 recycled per
            # chunk thereafter (next-chunk boundary prefetch)
            tc.tile_pool(name="xppool", bufs=20) as xppool,
            tc.tile_pool(name="apool", bufs=1) as apool,
            tc.tile_pool(name="tpool", bufs=4) as tpool,
            tc.tile_pool(name="bpool", bufs=2) as bpool,
            tc.tile_pool(name="biasp", bufs=D // 512) as biasp,
            tc.tile_pool(name="epool", bufs=10) as epool,
            # chunk-0 base results parked in SBUF until gate arrives
            tc.tile_pool(name="e0pool", bufs=MT) as e0pool,
            tc.tile_pool(name="rgw", bufs=16) as rgw,
            tc.tile_pool(name="rdram", bufs=1, space="DRAM") as rdram,
        ):
            ident = const_pool.tile([128, 128], f32)
            make_identity(nc, ident)
            gate32 = gate_pool.tile([ER, 1], f32)

            bias_all = []
            if has_bias:
                for hh in range(D // 512):
                    bias_bc = biasp.tile([128, 512], f32, tag="biasbc",
                                         name=f"biasbc_{hh}")
                    nc.sync.dma_start(
                        bias_bc[0:1, :],
                        b_row[:, hh * 512:(hh + 1) * 512])
                    nc.gpsimd.partition_broadcast(bias_bc[:],
                                                  bias_bc[0:1, :])
                    bias_all.append(bias_bc)

            # ====== router inputs ======================================
            wr_sb = rsb.tile([128, KT, NMOD], bf16)
            pt_sb = rsb.tile([128, 4, B], f32)
            msel_sb = rsb.tile([ER, E * B], f32)
            sc = rsb.tile([1, E * B], f32)
            a_sb = apool.tile([128, KT, ER], bf16)
            lg_sb = rsb.tile([NMOD, B], f32)

            def emit_router_partial_evict(wc_ps):
                """DVE copy of the router partial out of PSUM (emitted
                early so the later PE transposes never wait on it)."""
                wc_sb = rsb.tile([NMOD, DSH], f32)
                nc.vector.tensor_copy(wc_sb[:], wc_ps[:])
                return wc_sb

            def emit_router_tail(mps, wc_sb):
                """Router part-A tail: logits partial + AllReduce kick."""
                wct = rsb.tile([128, 4 * NMOD], f32)
                for j in range(4):
                    tp = mps.tile([128, NMOD], f32, tag="ps",
                                  name=f"tp_{j}")
                    nc.tensor.transpose(
                        tp[:], wc_sb[:, j * 128:(j + 1) * 128],
                        ident[0:NMOD, 0:NMOD])
                    nc.vector.tensor_copy(
                        wct[:, j * NMOD:(j + 1) * NMOD], tp[:])
                lg_ps = mps.tile([NMOD, B], f32, tag="ps", name="lg_ps")
                for j in range(4):
                    nc.tensor.matmul(lg_ps[:],
                                     wct[:, j * NMOD:(j + 1) * NMOD],
                                     pt_sb[:, j, :],
                                     start=(j == 0), stop=(j == 3))
                lp_sb = rsb.tile([NMOD, B], f32)
                nc.vector.tensor_copy(lp_sb[:], lg_ps[:])
                cc_in = rdram.tile([NMOD, B], f32)
                cc_out = rdram.tile([NMOD, B], f32)
                nc.gpsimd.dma_start(cc_in[:], lp_sb[:])
                nc.gpsimd.collective_compute(
                    "AllReduce", alu.add,
                    replica_groups=[list(range(N_CORES))],
                    ins=[cc_in.opt()], outs=[cc_out.opt()])
                nc.gpsimd.dma_start(lg_sb[:], cc_out[:])

                # collective-independent: expert ranks from scores
                rank = rsb.tile([1, E * B], f32)
                nc.vector.memset(rank[:], 0.0)
                tmp = rsb.tile([1, B], f32)
                for e in range(E):
                    re = rank[:, e * B:(e + 1) * B]
                    se = sc[:, e * B:(e + 1) * B]
                    for e2 in range(E):
                        if e2 == e:
                            continue
                        s2 = sc[:, e2 * B:(e2 + 1) * B]
                        nc.vector.tensor_tensor(tmp[:], s2, se, op=alu.is_gt)
                        nc.vector.tensor_add(re, re, tmp[:])
                        if e2 < e:
                            nc.vector.tensor_tensor(tmp[:], s2, se,
                                                    op=alu.is_equal)
                            nc.vector.tensor_add(re, re, tmp[:])
                w_hi = rsb.tile([1, E * B], f32)
                nc.vector.tensor_scalar(w_hi[:], rank[:], float(k),
                                        1.0 / float(k),
                                        op0=alu.is_lt, op1=alu.mult)
                w_lo = rsb.tile([1, E * B], f32)
                nc.vector.tensor_scalar(w_lo[:], rank[:], float(k_lo),
                                        1.0 / float(k_lo),
                                        op0=alu.is_lt, op1=alu.mult)
                diff = rsb.tile([1, E * B], f32)
                nc.vector.tensor_sub(diff[:], w_hi[:], w_lo[:])
                return w_lo, diff

            # ====== router part B ======================================
            def emit_part_b(mps, w_lo, diff):
                ltp = mps.tile([B, NMOD], f32, tag="ps", name="ltp")
                nc.tensor.transpose(ltp[:], lg_sb[:], ident[0:NMOD, 0:NMOD])
                lt = rsb.tile([B, NMOD], f32)
                nc.vector.tensor_copy(lt[:], ltp[:])
                mx = rsb.tile([B, 1], f32)
                nc.vector.tensor_reduce(out=mx[:], in_=lt[:], op=alu.max,
                                        axis=mybir.AxisListType.X)
                mxn = rsb.tile([B, 1], f32)
                nc.vector.tensor_scalar_mul(mxn[:], mx[:], -1.0)
                ex = rsb.tile([B, NMOD], f32)
                nc.scalar.activation(ex[:], lt[:], act_fn.Exp, bias=mxn[:])
                sm = rsb.tile([B, 1], f32)
                nc.vector.tensor_reduce(out=sm[:], in_=ex[:], op=alu.add,
                                        axis=mybir.AxisListType.X)
                rs = rsb.tile([B, 1], f32)
                nc.vector.reciprocal(rs[:], sm[:])
                p0 = rsb.tile([B, 1], f32)
                nc.vector.tensor_mul(
                    p0[:], ex[:, module_idx:module_idx + 1], rs[:])
                hi = rsb.tile([B, 1], f32)
                nc.vector.tensor_single_scalar(hi[:], p0[:], 0.5, alu.is_gt)
                hp = mps.tile([1, B], f32, tag="ps", name="hp")
                nc.tensor.transpose(hp[:], hi[:], ident[0:B, 0:B])
                hi_row = rsb.tile([1, B], f32)
                nc.vector.tensor_copy(hi_row[:], hp[:])
                gate = rsb.tile([1, E * B], f32)
                for e in range(E):
                    nc.vector.tensor_mul(gate[:, e * B:(e + 1) * B],
                                         diff[:, e * B:(e + 1) * B],
                                         hi_row[:])
                nc.vector.tensor_add(gate[:], gate[:], w_lo[:])
                gateb = rsb.tile([ER, E * B], f32)
                nc.gpsimd.partition_broadcast(gateb[:], gate[:])
                g32m = rsb.tile([ER, E * B], f32)
                nc.vector.tensor_tensor(g32m[:], gateb[:], msel_sb[:],
                                        op=alu.mult)
                nc.vector.tensor_reduce(out=gate32[:], in_=g32m[:],
                                        op=alu.add,
                                        axis=mybir.AxisListType.X)

            # ============== main: base + lora ==========================
            with (
                tc.tile_pool(name="mps", bufs=8, space="PSUM") as mps,
            ):
                CHUNKS = list(range(0, D, 512))
                tT_tiles = [None] * (MT // 4)
                e0_tiles = [None] * MT
                w_lo = diff = wc_sb = None
                GS = 4
                NG = MT // GS
                NXPRE = 8  # next-chunk xs tiles prefetched during mg3

                def store_out(ev, m, col0):
                    nc.scalar.dma_start(
                        out[m * 128:(m + 1) * 128, col0:col0 + 512], ev[:])

                def evict0(ps, m):
                    """Chunk 0: park base-only PSUM in SBUF (bf16).

                    On ACT: chunk-0's DVE stream carries the router
                    part-B chain (gated on the AllReduce), and the strict
                    FIFO would stall these PSUM-releasing copies behind
                    it."""
                    ev = e0pool.tile([128, 512], bf16, tag="e0",
                                     name=f"e0_{m}")
                    e0_tiles[m] = ev
                    if has_bias:
                        nc.vector.tensor_add(ev[:], ps[:], bias_all[0][:])
                    else:
                        nc.scalar.activation(ev[:], ps[:], act_fn.Copy)

                def evict_split(ps_lo, ps_hi, m, col0, c):
                    """lo+hi partial sums -> bf16 SBUF -> DRAM.

                    PSUM release is pure-DVE (copy hi, then add lo
                    in-place): staging through ACT couples it to the
                    store FIFO and stalls the next chunk's matmuls."""
                    ev = epool.tile([128, 512], bf16, tag="ev",
                                    name=f"ev_{c}_{m}")
                    nc.vector.tensor_copy(ev[:], ps_hi[:])
                    if has_bias:
                        nc.vector.tensor_add(ev[:], ev[:],
                                             bias_all[col0 // 512][:])
                    nc.vector.tensor_add(ev[:], ps_lo[:], ev[:])
                    store_out(ev, m, col0)

                def emit_c0_combines(ms, b_scl0):
                    """Chunk-0 lora combine for m-tiles `ms` (spread over
                    chunk 1's group boundaries), 4-way row-tiled."""
                    lps = []
                    for j, m in enumerate(ms):
                        tsl = tT_tiles[m // 4][j * ER:(j + 1) * ER,
                                               (m % 4) * 128:
                                               (m % 4) * 128 + 128]
                        lp = mps.tile([128, 512], f32, tag="ps",
                                      name=f"lp_{m}")
                        nc.tensor.matmul(lp[:], tsl,
                                         b_scl0[j * ER:(j + 1) * ER, :],
                                         start=True, stop=True,
                                         tile_position=(j * ER, 0))
                        lps.append(lp)
                    for lp, m in zip(lps, ms):
                        ev = epool.tile([128, 512], bf16, tag="ev",
                                        name=f"ev_c0_{m}")
                        nc.vector.tensor_add(ev[:], lp[:], e0_tiles[m][:])
                        store_out(ev, m, 0)

                # ===== upfront loads: everything chunk-0 mg0 needs, on
                # three separate rings in consumption order ==============
                xs_pref = {}
                for kt_n in range(KT):
                    xs0 = xppool.tile([128, GS * 128], bf16, tag="xp",
                                      name=f"xp_0_{kt_n}")
                    nc.sync.dma_start(
                        xs0[:], xT[kt_n * 128:(kt_n + 1) * 128,
                                   0:GS * 128])
                    xs_pref[kt_n] = xs0
                w0_tiles = []
                for kt_n in range(KT):
                    wt = w0pool.tile([128, 512], bf16, tag="w0",
                                     name=f"w0_{kt_n}")
                    nc.scalar.dma_start(
                        wt[:], WbT[kt_n * 128:(kt_n + 1) * 128, 0:512])
                    w0_tiles.append(wt)
                nc.gpsimd.dma_start(a_sb[:], A_rhs[:])
                nc.gpsimd.dma_start(wr_sb[:], WrT[:])
                gwt_tiles = []
                for kt_n in range(KT):
                    gwt = rgw.tile([128, DSH], bf16, tag="gwt",
                                   name=f"gwt_{kt_n}")
                    nc.gpsimd.dma_start(
                        gwt[:], gw[kt_n * 128:(kt_n + 1) * 128, :])
                    gwt_tiles.append(gwt)
                nc.gpsimd.dma_start(pt_sb[:], pooledT[:])
                nc.gpsimd.dma_start(msel_sb[:], msel[:])
                nc.gpsimd.dma_start(sc[:], scores_f[:])

                prefetched = None
                b_scl0 = None
                for c, col0 in enumerate(CHUNKS):
                    wtiles = prefetched  # [4 x [128,8,512]] for c>=1

                    def prefetch_w8(j, c=c, col_next=(
                            CHUNKS[c + 1] if c + 1 < len(CHUNKS) else 0)):
                        wt = wpool.tile([128, 8, 512], bf16, tag="w",
                                        name=f"w_{c + 1}_{j}")
                        nc.scalar.dma_start(
                            wt[:],
                            WbT[j * 1024:(j + 1) * 1024,
                                col_next:col_next + 512]
                            .rearrange("(a p) m -> p a m", p=128))
                        prefetched[j] = wt

                    def prefetch_x(kt_n, c=c):
                        xs = xppool.tile([128, GS * 128], bf16, tag="xp",
                                         name=f"xp_{c + 1}_{kt_n}")
                        nc.sync.dma_start(
                            xs[:], xT[kt_n * 128:(kt_n + 1) * 128,
                                      0:GS * 128])
                        xs_pref[kt_n] = xs

                    def w_lo_ap(kt):
                        if c == 0:
                            return w0_tiles[kt][0:64, :]
                        return wtiles[kt // 8][0:64, kt % 8, :]

                    def w_hi_ap(kt):
                        if c == 0:
                            return w0_tiles[kt][64:128, :]
                        return wtiles[kt // 8][64:128, kt % 8, :]

                    if c + 1 < len(CHUNKS):
                        prefetched = [None] * 4
                        n_pref = 4
                    else:
                        n_pref = 0

                    # B slice scaled by this core's gate, replicated to 4
                    # partition blocks for the concurrent combines
                    b_stg = bpool.tile([ER, 512], f32, tag="bstg",
                                       name=f"bstg_{c}")
                    nc.gpsimd.dma_start(b_stg[:],
                                        B_cat[:, col0:col0 + 512])
                    if c == 0:
                        bstg0 = b_stg
                        bh = None
                    else:
                        bh = bpool.tile([128, 512], bf16, tag="bscl",
                                        name=f"bscl_{c}")
                        nc.vector.tensor_scalar_mul(bh[0:ER, :], b_stg[:],
                                                    gate32[:, 0:1])
                        for j in range(1, 4):
                            nc.gpsimd.dma_start(
                                bh[j * ER:(j + 1) * ER, :], bh[0:ER, :])

                    for mg in range(NG):
                        if c == 0:
                            pss = [mps.tile([128, 512], f32, tag="ps",
                                            name=f"ps_{c}_{mg}_{mi}")
                                   for mi in range(GS)]
                            ps_t = mps.tile([ER, 512], f32, tag="ps",
                                            name=f"pst_{mg}")
                            if mg == 0:
                                wc_ps = mps.tile([NMOD, DSH], f32,
                                                 tag="ps", name="wc_ps")
                        else:
                            pss = [(mps.tile([128, 512], f32, tag="ps",
                                             name=f"pl_{c}_{mg}_{mi}"),
                                    mps.tile([128, 512], f32, tag="ps",
                                             name=f"ph_{c}_{mg}_{mi}"))
                                   for mi in range(GS)]
                        for kt in range(KT):
                            if mg == 0 and kt in xs_pref:
                                xs = xs_pref.pop(kt)
                            else:
                                xs = xpool.tile([128, GS * 128], bf16,
                                                tag="x",
                                                name=f"x_{c}_{mg}_{kt}")
                                nc.sync.dma_start(
                                    xs[:],
                                    xT[kt * 128:(kt + 1) * 128,
                                       mg * GS * 128:(mg + 1) * GS * 128])
                            if mg >= 1:
                                slot = (mg - 1) * KT + kt
                                if slot % 16 == 0 and slot // 16 < n_pref:
                                    prefetch_w8(slot // 16)
                                if (mg == NG - 1 and kt % 4 == 0
                                        and kt // 4 < NXPRE
                                        and c + 1 < len(CHUNKS)):
                                    prefetch_x(kt // 4)
                            if c == 0 and mg == 1 and kt == 0:
                                # stage the router partial out of PSUM
                                # early (DVE) so the mg1-end PE transposes
                                # have no cross-engine wait
                                wc_sb = emit_router_partial_evict(wc_ps)
                            last = kt == KT - 1
                            for mi in range(GS):
                                m = mg * GS + mi
                                xsl = xs[:, mi * 128:(mi + 1) * 128]
                                if c == 0:
                                    nc.tensor.matmul(
                                        pss[mi][:], xsl,
                                        w0_tiles[kt][:],
                                        start=(kt == 0), stop=False)
                                    if last:
                                        evict0(pss[mi], m)
                                else:
                                    lo, hi = pss[mi]
                                    nc.tensor.matmul(
                                        lo[:], xsl[0:64, :],
                                        w_lo_ap(kt),
                                        start=(kt == 0), stop=False,
                                        tile_position=(0, 0))
                                    nc.tensor.matmul(
                                        hi[:], xsl[64:128, :],
                                        w_hi_ap(kt),
                                        start=(kt == 0), stop=last,
                                        tile_position=(64, 0))
                            if c != 0 and last:
                                # 4 K=32 lora-combine matmuls on distinct
                                # 32-row PE groups -> concurrent
                                for mi in range(GS):
                                    m = mg * GS + mi
                                    lo, hi = pss[mi]
                                    tsl = tT_tiles[m // 4][
                                        mi * ER:(mi + 1) * ER,
                                        (m % 4) * 128:(m % 4) * 128 + 128]
                                    nc.tensor.matmul(
                                        lo[:], tsl,
                                        bh[mi * ER:(mi + 1) * ER, :],
                                        start=False, stop=True,
                                        tile_position=(mi * ER, 0))
                                for mi in range(GS):
                                    lo, hi = pss[mi]
                                    evict_split(lo, hi, mg * GS + mi,
                                                col0, c)
                            if c == 0:
                                nc.tensor.matmul(
                                    ps_t[:], a_sb[:, kt, :], xs[:],
                                    start=(kt == 0), stop=(kt == KT - 1))
                            if c == 0 and mg == 0:
                                nc.tensor.matmul(wc_ps[:],
                                                 wr_sb[:, kt, :],
                                                 gwt_tiles[kt][:],
                                                 start=(kt == 0),
                                                 stop=(kt == KT - 1))
                        if c == 0:
                            # t for this m-group, replicated to 4
                            # partition blocks for the concurrent combines
                            tT = tpool.tile([128, 512], bf16, tag="tT",
                                            name=f"tT_{mg}")
                            nc.vector.tensor_copy(tT[0:ER, :], ps_t[:])
                            for j in range(1, 4):
                                nc.gpsimd.dma_start(
                                    tT[j * ER:(j + 1) * ER, :],
                                    tT[0:ER, :])
                            tT_tiles[mg] = tT
                        if c == 0 and mg == 1:
                            # router logits partial + AllReduce kick-off
                            # (group boundary: mid-group emission would
                            # deadlock the PSUM pool rotation)
                            w_lo, diff = emit_router_tail(mps, wc_sb)
                        if c == 0 and mg == 3:
                            emit_part_b(mps, w_lo, diff)
                            b_scl0 = bpool.tile([128, 512], bf16,
                                                tag="bscl", name="bscl_0_0")
                            nc.vector.tensor_scalar_mul(
                                b_scl0[0:ER, :], bstg0[:], gate32[:, 0:1])
                            for j in range(1, 4):
                                nc.gpsimd.dma_start(
                                    b_scl0[j * ER:(j + 1) * ER, :],
                                    b_scl0[0:ER, :])
                        if c == 1:
                            # chunk-0 lora combine, 4 m-tiles per group
                            # boundary (gate landed during chunk 0)
                            emit_c0_combines(range(4 * mg, 4 * mg + 4),
                                             b_scl0)

    nc.compile()
    return nc


def kernel(**inputs) -> np.ndarray:
    _install_profile_hook()

    x = np.asarray(inputs["x"], dtype=np.float32)
    expert_scores = np.asarray(inputs["expert_scores"], dtype=np.float32)
    W_base = np.asarray(inputs["W_base"], dtype=np.float32)
    b_base = np.asarray(inputs["b_base"], dtype=np.float32)
    gating_W = np.asarray(inputs["gating_W"], dtype=np.float32)
    W_r = np.asarray(inputs["W_r"], dtype=np.float32)
    lora_A = np.asarray(inputs["lora_A"], dtype=np.float32)
    lora_B = np.asarray(inputs["lora_B"], dtype=np.float32)
    module_idx = int(np.asarray(inputs["module_idx"]))
    k = int(np.asarray(inputs["k"]))

    has_bias = bool(np.any(b_base != 0.0))
    key = (k, module_idx, has_bias)
    if key not in _PROGRAM_CACHE:
        _PROGRAM_CACHE[key] = _build_program(k, module_idx, has_bias)
    nc = _PROGRAM_CACHE[key]

    # --- host-side layout prep (transposes/slices/bf16 rounding) --------
    WbT_np = _to_bf16(W_base.T)                          # [D, D]
    # packed [128, KT*NMOD]: row p holds W_r.T[kt*128+p, :] for each kt
    WrT_np = _to_bf16(W_r.T.reshape(KT, 128, NMOD)
                      .transpose(1, 0, 2).reshape(128, KT * NMOD))
    A_np = _to_bf16(lora_A.reshape(ER, D).T.reshape(KT, 128, ER)
                    .transpose(1, 0, 2).reshape(128, KT * ER))
    B_np = np.ascontiguousarray(
        lora_B.transpose(0, 2, 1).reshape(ER, D))        # [ER, D] fp32
    scores_f_np = np.ascontiguousarray(
        expert_scores.T.reshape(1, E * B))               # [1, E*B]
    b_row_np = b_base.reshape(1, D)
    pooled = x[:, -1, :]                                 # [B, D]

    in_maps = []
    for c in range(N_CORES):
        msel_np = np.zeros((ER, E, B), dtype=np.float32)
        for p in range(ER):
            msel_np[p, p // R, c] = 1.0
        msel_np = msel_np.reshape(ER, E * B)
        in_maps.append({
            "xT": _to_bf16(x[c].T),
            "WbT": WbT_np,
            "gw": _to_bf16(gating_W[:, c * DSH:(c + 1) * DSH]),
            "WrT": WrT_np,
            "pooledT": np.ascontiguousarray(
                pooled[:, c * DSH:(c + 1) * DSH].T.reshape(4, 128, B)
                .transpose(1, 0, 2).reshape(128, 4 * B)),
            "scores_f": scores_f_np,
            "A_rhs": A_np,
            "B_cat": B_np,
            "b_row": b_row_np,
            "msel": msel_np,
        })

    from concourse.bass_utils import run_bass_kernel_spmd
    res = run_bass_kernel_spmd(nc, in_maps, core_ids=list(range(N_CORES)))
    return np.stack([np.asarray(res.results[c]["out"])
                     .astype(np.float32) for c in range(N_CORES)], axis=0)


if __name__ == "__main__":
    rng = np.random.default_rng(0)
    demo = {
        "x": (rng.standard_normal((B, L, D)) * 0.02).astype(np.float32),
        "expert_scores": rng.random((B, E), dtype=np.float32),
        "W_base": (rng.standard_normal((D, D)) * 0.02).astype(np.float32),
        "b_base": np.zeros(D, np.float32),
        "gating_W": (rng.standard_normal((D, D)) * 0.02).astype(np.float32),
        "W_r": (rng.standard_normal((NMOD, D)) * 0.02).astype(np.float32),
        "lora_A": (rng.standard_normal((E, R, D)) * 0.02).astype(np.float32),
        "lora_B": (rng.standard_normal((E, D, R)) * 0.02).astype(np.float32),
        "module_idx": 0,
        "k": 2,
    }
    y = kernel(**demo)
    print("out", y.shape, y.dtype, float(np.abs(y).max()))
